# revision 46
# baseline (speedup 1.0000x reference)
"""Trainium2 Bass/Tile kernel for nn_Attention_3418793967804.

8-way data parallel over batch (1 batch per NeuronCore). Per core:
qkv 1x1 conv (+folded BN), 4-head attention over 2304 positions,
depthwise 3x3 conv on v, residual add, final 1x1 conv (+folded BN).

Layout: S^T score tiles (keys on partitions) via row-packed K=32 bf16
matmuls; exp on the scalar engine PSUM->SBUF; attention-value matmuls
use a [V^T | ones-col] 65-wide stationary per head so one matmul yields
the numerator (rows 0-63) and the softmax denominator (row 64) with no
separate denominator matmuls and minimal PE-array power (the device is
power-throttled, so idle-row activity costs real time); normalization
stages the denominator to one sbuf partition, reciprocals it, and
gpsimd-broadcasts it across 64 partitions; the depthwise 3x3 conv stays
on the PE as 9 diagonal matmuls (the mostly-zero stationary switches
little, while DVE mac chains burn real power out of the shared throttle
budget); dw/c2 chunks ride freed PSUM slots at i-chunk transitions;
both conv biases folded off the PE (dw bias through the final conv on
the host, final bias into the DVE evacuation).
"""
import numpy as np

import concourse.bass as bass
import concourse.mybir as mybir
import concourse.tile as tile
from concourse import bacc

F32 = mybir.dt.float32
F32R = mybir.dt.float32r
BF16 = mybir.dt.bfloat16
EXP = mybir.ActivationFunctionType.Exp

CH = 256
HW = 2304
H = W = 48
NH = 4
DK = 32
DH = 64
SCALE = float(DK) ** -0.5
EPS = 1e-3

IC_SIZES = [512, 512, 512, 512, 256]
IC_STARTS = [0, 512, 1024, 1536, 2048]
JB = 18          # 2304 / 128 j-blocks
QN = 384         # qkv/dwconv spatial chunk = 8 rows of 48
NQ = HW // QN    # 6
PW = 50          # padded width/height


def build_consts(qkv_w, qkv_g, qkv_b, qkv_m, qkv_v, c1_w, c1_g, c1_b, c1_m, c1_v,
                 c2_w, c2_g, c2_b, c2_m, c2_v):
    """Fold BN into conv weights and pack into device-layout numpy arrays."""
    f = np.float32
    sq = qkv_g / np.sqrt(qkv_v + EPS)
    Wq = (qkv_w[:, :, 0, 0] * sq[:, None]).astype(f)       # (512, 256)
    bq = (qkv_b - qkv_m * sq).astype(f)                    # (512,)
    s1 = c1_g / np.sqrt(c1_v + EPS)
    W1 = (c1_w[:, 0, :, :] * s1[:, None, None]).astype(f)  # (256, 3, 3)
    b1 = (c1_b - c1_m * s1).astype(f)
    s2 = c2_g / np.sqrt(c2_v + EPS)
    W2 = (c2_w[:, :, 0, 0] * s2[:, None]).astype(f)        # (256, 256)
    b2 = (c2_b - c2_m * s2).astype(f)

    # qkv output channel permutation: cols 0-127 Q_all (h*32+dk), 128-255 K_all,
    # 256-511 v in natural c = h*64+d order
    perm = np.zeros(512, dtype=np.int64)
    for col in range(128):
        h, dk = col // 32, col % 32
        perm[col] = 128 * h + dk
        perm[128 + col] = 128 * h + 32 + dk
    for col in range(256):
        h, d = col // 64, col % 64
        perm[256 + col] = 128 * h + 64 + d
    wt = np.ascontiguousarray(Wq[perm].T)                  # (256 ic, 512 col)
    bqkv = np.zeros((128, 4), f)
    for occ in range(4):
        bqkv[:, occ] = bq[perm[occ * 128:(occ + 1) * 128]]

    # depthwise conv diagonal stationaries: block (chunk, tap) at cols
    # (chunk*9+tap)*128, diag entries W1[chunk*128 + c, tap]
    diag = np.zeros((128, 18 * 128), f)
    for chunk in range(2):
        for tap in range(9):
            di, dj = tap // 3, tap % 3
            base = (chunk * 9 + tap) * 128
            idx = np.arange(128)
            diag[idx, base + idx] = W1[chunk * 128 + idx, di, dj]

    # per-partition dw tap weights for the gpsimd chain: col chunk*9+tap
    w1t = np.zeros((128, 18), f)
    for chunk in range(2):
        for tap in range(9):
            w1t[:, chunk * 9 + tap] = W1[chunk * 128:(chunk + 1) * 128,
                                         tap // 3, tap % 3]

    import ml_dtypes
    ident = np.eye(128, dtype=np.float32).astype(ml_dtypes.bfloat16)
    w2t = np.ascontiguousarray(W2.T)                        # (256 c, 256 oc)
    # the dw bias passes linearly through the final conv: fold it there
    b2e = (b2 + W2 @ b1).astype(f)
    b2p = np.stack([b2e[0:128], b2e[128:256]], axis=1).astype(f)  # (128, 2)
    return dict(wt=wt, bqkv=bqkv, diag=diag, w1t=w1t, w2t=w2t, ident=ident,
                b2p=b2p)


def build_nc(debug=False):
    nc = bacc.Bacc("TRN2", target_bir_lowering=False, debug=False,
                   enable_asserts=True, num_devices=8)
    dp = {}
    def din(name, shape, dt=F32):
        dp[name] = nc.dram_tensor(name, list(shape), dt, kind="ExternalInput").ap()
    din("x", (256, HW), F32R)
    din("wt", (256, 512), F32R)
    din("bqkv", (128, 4))
    din("diag", (128, 18 * 128), F32R)
    din("w1t", (128, 18))
    din("w2t", (256, 256), F32R)
    din("b2p", (128, 2))
    din("ident", (128, 128), BF16)
    out_d = nc.dram_tensor("out", [256, HW], F32, kind="ExternalOutput").ap()
    dbg = {}
    if debug:
        for name, shape in [("dq", (128, HW)), ("dk", (128, HW)),
                            ("dvt", (128, JB * 512)), ("dvp0", (128, PW * PW)),
                            ("dvp1", (128, PW * PW)), ("dy0", (128, HW)),
                            ("dy1", (128, HW)), ("dot0", (128, HW)), ("dot1", (128, HW)),
                            ("dp2", (128, 1024))]:
            dbg[name] = nc.dram_tensor(name, list(shape), F32, kind="ExternalOutput").ap()

    with tile.TileContext(nc) as tc:
        build_body(nc, tc, dp, out_d, dbg)
    nc.compile()
    return nc


def build_body(nc, tc, dp, out_d, dbg):
    from contextlib import ExitStack
    with ExitStack() as ctx:
        ep = ctx.enter_context
        wpool = ep(tc.tile_pool(name="w", bufs=1))
        xpool = ep(tc.tile_pool(name="x", bufs=1))
        qkpool = ep(tc.tile_pool(name="qk", bufs=1))
        vtpool = ep(tc.tile_pool(name="vt", bufs=1))
        vppool = ep(tc.tile_pool(name="vp", bufs=1))
        ypool = ep(tc.tile_pool(name="y", bufs=1))
        ppool = ep(tc.tile_pool(name="pp", bufs=8))
        npool = ep(tc.tile_pool(name="np", bufs=1))
        otpool = ep(tc.tile_pool(name="ot", bufs=1))
        obpool = ep(tc.tile_pool(name="ob", bufs=3))

        # --- weights & inputs ---
        wt_r = [wpool.tile([128, 512], F32R, tag=f"wt{c}", name=f"wt{c}") for c in range(2)]
        diag_r = wpool.tile([128, 18 * 128], F32R, tag="diag", name="diag")
        w1t_f = wpool.tile([128, 18], F32, tag="w1t", name="w1t")
        w2t_r = [wpool.tile([128, 256], F32R, tag=f"w2t{c}", name=f"w2t{c}") for c in range(2)]
        bq_f = wpool.tile([128, 4], F32, tag="bqf", name="bqf")
        b2_f = wpool.tile([128, 2], F32, tag="b2f", name="b2f")
        x_r = [xpool.tile([128, HW], F32R, tag=f"x{c}", name=f"x{c}") for c in range(2)]

        # critical-path loads first (each dma issue costs ~650ns serialized on
        # the sync queue): qkv bias + first x quarter of both halves + wt --
        # exactly what the first qkv matmul needs -- then the remaining x
        # quarters and the late-use small consts
        qr_sl = lambda qr: slice(qr * (HW // 4), (qr + 1) * (HW // 4))
        nc.sync.dma_start(bq_f[:], dp["bqkv"][:])
        for c in range(2):
            nc.sync.dma_start(x_r[c][:, qr_sl(0)], dp["x"][128 * c:128 * (c + 1), qr_sl(0)])
        for c in range(2):
            nc.sync.dma_start(wt_r[c][:], dp["wt"][128 * c:128 * (c + 1), :])
        for qr in range(1, 4):
            for c in range(2):
                nc.sync.dma_start(x_r[c][:, qr_sl(qr)], dp["x"][128 * c:128 * (c + 1), qr_sl(qr)])
        nc.sync.dma_start(b2_f[:], dp["b2p"][:])
        nc.sync.dma_start(w1t_f[:], dp["w1t"][:])

        Q = qkpool.tile([128, HW], BF16, tag="Q", name="Q")
        K = qkpool.tile([128, HW], BF16, tag="K", name="K")
        # per (jb, head) stationary block: [V^T_h (64 cols) | ones (1 col)];
        # the 65th output row collects the softmax denominator at 1/128th of
        # the array power a 64-wide ones block would burn
        VTO = vtpool.tile([128, JB * 512], BF16, tag="VTO", name="VTO")
        vto4 = VTO[:].rearrange("p (b k) -> p b k", k=128)
        nc.gpsimd.memset(vto4[:, :, 64:65], 1.0)
        if dbg:
            # cols 65-127 of each block are unused; init them so the debug
            # full-tile DMA dump doesn't read uninitialized sbuf
            nc.gpsimd.memset(vto4[:, :, 65:128], 0.0)
        id_b = vtpool.tile([128, 128], BF16, tag="idb", name="idb")
        nc.sync.dma_start(id_b[:], dp["ident"][:])
        vp = [vppool.tile([128, PW * PW], F32R, tag=f"vp{c}", name=f"vp{c}") for c in range(2)]
        vf = [vppool.tile([128, HW], BF16, tag=f"vf{c}", name=f"vf{c}") for c in range(2)]
        for c in range(2):
            # zero only the 1-wide pad border; the interior is fully
            # overwritten by the qkv V evacuation
            vp3i = vp[c][:].bitcast(F32).rearrange("p (r w) -> p r w", w=PW)
            nc.gpsimd.memset(vp3i[:, 0:1, :], 0.0)
            nc.gpsimd.memset(vp3i[:, 49:50, :], 0.0)
            nc.gpsimd.memset(vp3i[:, 1:49, 0:1], 0.0)
            nc.gpsimd.memset(vp3i[:, 1:49, 49:50], 0.0)
        y_all = [ypool.tile([128, HW], F32, tag=f"y{c}", name=f"y{c}") for c in range(2)]
        ot = [otpool.tile([128, HW], F32R, tag=f"ot{c}", name=f"ot{c}") for c in range(2)]
        last_p2 = [None]
        vf_w = {}

        with tc.tile_pool(name="psS", bufs=2, space="PSUM") as psS, \
             tc.tile_pool(name="psU", bufs=1, space="PSUM") as psU:

            def emit_qkv(occ, g):
                # one 384-wide chunk of the qkv projection for output group occ
                ps = psS.tile([128, 1024], F32, tag="s2", name="s2")
                sl = slice(g * QN, (g + 1) * QN)
                for c in range(2):
                    nc.tensor.matmul(
                        ps[:, 0:QN], wt_r[c][:, occ * 128:(occ + 1) * 128],
                        x_r[c][:, sl], start=(c == 0), stop=(c == 1))
                bias_ap = bq_f[:, occ:occ + 1]
                if occ == 0:
                    nc.vector.tensor_scalar_add(Q[:, sl], ps[:, 0:QN], bias_ap)
                elif occ == 1:
                    nc.vector.tensor_scalar_add(K[:, sl], ps[:, 0:QN], bias_ap)
                else:
                    c = occ - 2
                    # vf first: the V^T transposes on the PE wait only on vf,
                    # so don't queue it behind the padded-image write
                    vf_w[(c, g)] = nc.vector.tensor_scalar_add(
                        vf[c][:, sl], ps[:, 0:QN], bias_ap)
                    vp3 = vp[c][:].rearrange("p (r w) -> p r w", w=PW)
                    dst = vp3[:, 1 + 8 * g:1 + 8 * g + 8, 1:49]
                    srcp = ps[:, 0:QN].rearrange("p (r w) -> p r w", w=48)
                    nc.vector.tensor_scalar_add(dst, srcp, bias_ap)

            def emit_dw(c, g, slot=None):
                # depthwise 3x3 conv chunk via 9 diagonal matmuls + bias row,
                # then add the attention output (PE fallback, used in the tail)
                ps = slot() if slot else psS.tile([128, 1024], F32, tag="s2", name="s2")
                vp3 = vp[c][:].rearrange("p (r w) -> p r w", w=PW)
                for tap in range(9):
                    di, dj = tap // 3, tap % 3
                    mov = vp3[:, 8 * g + di:8 * g + di + 8, dj:dj + 48]
                    nc.tensor.matmul(
                        ps[:, 0:QN], diag_r[:, (c * 9 + tap) * 128:(c * 9 + tap + 1) * 128],
                        mov, start=(tap == 0), stop=(tap == 8))
                sl = slice(g * QN, (g + 1) * QN)
                nc.vector.tensor_add(ot[c][:, sl], ps[:, 0:QN], y_all[c][:, sl])

            MUL = mybir.AluOpType.mult
            ADD = mybir.AluOpType.add

            def emit_dw_pool(c, g):
                # depthwise 3x3 conv chunk as a 9-step multiply-accumulate
                # chain off the PE: keeps the full 128x128 array from burning
                # power on diagonal matmuls (the device is power-throttled)
                sl = slice(g * QN, (g + 1) * QN)
                vp3 = vp[c][:].rearrange("p (r w) -> p r w", w=PW)
                acc = y_all[c][:, sl]
                for tap in range(9):
                    di, dj = tap // 3, tap % 3
                    win = vp3[:, 8 * g + di:8 * g + di + 8, dj:dj + 48]
                    nc.vector.scalar_tensor_tensor(
                        ot[c][:, sl], win, w1t_f[:, c * 9 + tap:c * 9 + tap + 1],
                        acc, MUL, ADD)
                    acc = ot[c][:, sl]

            # minimal qkv pre-work: just what the first attention iterations
            # need; the rest interleaves into ic 0 via qkv_sched
            for occ, g in [(0, 0), (0, 1), (1, 0), (2, 0), (3, 0)]:
                emit_qkv(occ, g)
            qkv_sched = {
                0: [(1, 1)], 1: [(2, 1), (3, 1)], 2: [(0, 2)],
                3: [(1, 2)], 4: [(2, 2), (3, 2)], 5: [(0, 3)],
                6: [(1, 3)], 7: [(2, 3), (3, 3)], 8: [(0, 4)],
                9: [(1, 4)], 10: [(2, 4), (3, 4)], 11: [(0, 5)],
                12: [(1, 5)], 13: [(2, 5), (3, 5)],
            }
            def emit_vtT(jb):
                # V^T 128x128 block transposes on the PE (bf16, via identity);
                # one strided DVE copy splits the two heads into their
                # [V^T | ones] stationary blocks
                for c in range(2):
                    ps = psS.tile([128, 1024], F32, tag="s2", name="s2")
                    nc.tensor.transpose(ps[:, 0:64].bitcast(BF16),
                                        vf[c][:, jb * 128:(jb + 1) * 128], id_b[:])
                    src = ps[:, 0:64].bitcast(BF16).rearrange(
                        "p (b k) -> p b k", k=64)
                    nc.vector.tensor_copy(
                        vto4[:, jb * 4 + 2 * c:jb * 4 + 2 * c + 2, 0:64], src)

            def emit_c2(occ, g, slot=None):
                # 384-col chunks aligned to the dw g-chunks so each c2 chunk
                # is ready as soon as its dw chunk lands; only g=5 remains in
                # the tail
                isl2 = slice(g * QN, (g + 1) * QN)
                ps = slot() if slot else psS.tile([128, 1024], F32, tag="s2", name="s2")
                for c in range(2):
                    nc.tensor.matmul(ps[:, 0:QN],
                                     w2t_r[c][:, occ * 128:(occ + 1) * 128],
                                     ot[c][:, isl2], start=(c == 0), stop=(c == 1))
                ob = obpool.tile([128, 512], F32, tag="ob", name="ob")
                nc.vector.tensor_scalar_add(ob[:, 0:QN], ps[:, 0:QN],
                                            b2_f[:, occ:occ + 1])
                nc.sync.dma_start(out_d[occ * 128:(occ + 1) * 128, isl2], ob[:, 0:QN])

            # ic -> dw/c2 chunks emitted at the END of that ic (after its
            # normalization frees the U psum slots); deps: dw(c,g) needs
            # y cols <= IC_STARTS[ic+1], c2(occ,g) needs dw chunk g done.
            # dw stays on the PE: the diagonal stationary is mostly zeros
            # (little switching power) while DVE chains burn real power out
            # of the shared throttle budget.
            pool_dw_sched = {}
            trans_sched = {
                0: [('dw', 0, 0), ('dw', 1, 0)],
                1: [('dw', 0, 1), ('dw', 1, 1), ('c2', 0, 0), ('c2', 1, 0)],
                2: [('dw', 0, 2), ('dw', 1, 2), ('dw', 0, 3), ('dw', 1, 3),
                    ('c2', 0, 1), ('c2', 1, 1)],
                3: [('dw', 0, 4), ('dw', 1, 4), ('c2', 0, 2), ('c2', 1, 2),
                    ('c2', 0, 3), ('c2', 1, 3), ('c2', 0, 4), ('c2', 1, 4)],
            }
            for ic in range(5):
                n = IC_SIZES[ic]
                i0 = IC_STARTS[ic]
                isl = slice(i0, i0 + n)
                Uh = [psU.tile([128, 512], F32, tag=f"Uh{h}", name=f"Uh{h}") for h in range(4)]

                def emit_qk_exp(jb):
                    jsl = slice(jb * 128, (jb + 1) * 128)
                    p2s = []
                    for hp in range(2):
                        s2 = psS.tile([128, 1024], F32, tag="s2", name="s2")
                        for hh in range(2):
                            h = 2 * hp + hh
                            nc.tensor.matmul(
                                s2[:, hh * 512:hh * 512 + n],
                                K[32 * h:32 * (h + 1), jsl],
                                Q[32 * h:32 * (h + 1), isl],
                                start=True, stop=True, tile_position=(32 * h, 0))
                        p2 = ppool.tile([128, 1024], BF16, tag="p2", name="p2")
                        last_p2[0] = p2
                        if n == 512:
                            nc.scalar.activation(p2[:], s2[:], EXP, scale=SCALE)
                        else:
                            s3 = s2[:].rearrange("p (a b) -> p a b", b=512)[:, :, 0:n]
                            p3 = p2[:].rearrange("p (a b) -> p a b", b=512)[:, :, 0:n]
                            nc.scalar.activation(p3, s3, EXP, scale=SCALE)
                        p2s.append(p2)
                    return p2s

                def emit_av(jb, p2s):
                    # one matmul per head: [V^T | ones-col] stationary gives
                    # numerator rows 0-63 and the denominator on row 64
                    for h in range(4):
                        hp, hh = h // 2, h % 2
                        mov = p2s[hp][:, hh * 512:hh * 512 + n]
                        nc.tensor.matmul(
                            Uh[h][0:65, 0:n],
                            VTO[:, jb * 512 + 128 * h:jb * 512 + 128 * h + 65],
                            mov, start=(jb == 0), stop=(jb == JB - 1))

                # 1-deep software pipeline: AV of jb-1 lands after QK+exp of jb
                prev = None
                for jb in range(JB):
                    p2s = emit_qk_exp(jb)
                    if prev is not None:
                        emit_av(jb - 1, prev)
                    prev = p2s
                    if ic == 0:
                        for occ, g in qkv_sched.get(jb, ()):
                            emit_qkv(occ, g)
                        emit_vtT(jb)
                    if (ic, jb) == (0, 5):
                        # late-use weight loads, clear of the transpose burst
                        for c in range(2):
                            nc.sync.dma_start(w2t_r[c][:], dp["w2t"][128 * c:128 * (c + 1), :])
                        nc.sync.dma_start(diag_r[:], dp["diag"][:])
                emit_av(JB - 1, prev)

                # normalization: y_h = U[0:64] / U[64] -- stage the denominator
                # row to sbuf (the custom-DVE reciprocal mishandles
                # partition-offset inputs, plain copies don't), take the
                # reciprocal on one partition, replicate it across 64
                # partitions on gpsimd, then multiply. Emit stage-by-stage
                # across all 4 heads so the DVE never sits waiting for a
                # Pool broadcast round-trip between heads.
                rbs = [npool.tile([128, 1024], F32, tag=f"rb{h}", name=f"rb{h}")
                       for h in range(4)]
                bcs = [npool.tile([128, 512], F32, tag=f"bc{h}", name=f"bc{h}")
                       for h in range(4)]
                for h in range(4):
                    nc.vector.tensor_copy(rbs[h][0:1, 512:512 + n],
                                          Uh[h][64:65, 0:n])
                for h in range(4):
                    nc.vector.reciprocal_approx_fast(rbs[h][0:1, 0:n],
                                                     rbs[h][0:1, 512:512 + n])
                for h in range(4):
                    nc.gpsimd.partition_broadcast(bcs[h][0:64, 0:n],
                                                  rbs[h][0:1, 0:n])
                for h in range(4):
                    hp, hh = h // 2, h % 2
                    nc.vector.tensor_mul(y_all[hp][64 * hh:64 * hh + 64, isl],
                                         Uh[h][0:64, 0:n], bcs[h][0:64, 0:n])

                # dw chains for completed y columns onto the gpsimd queue
                for g in pool_dw_sched.get(ic, []):
                    for c in range(2):
                        emit_dw_pool(c, g)

                # c2 chunks ride the just-freed U psum slots so they stay
                # out of the S-tile rotation (no ACT starvation)
                slots = [lambda h=h: psU.tile([128, 512], F32, tag=f"Uh{h}",
                                              name=f"aux{h}") for h in range(4)]
                for i, (kind, a, b_) in enumerate(trans_sched.get(ic, [])):
                    slot = slots[i % 4]
                    if kind == 'dw':
                        emit_dw(a, b_, slot=slot)
                    else:
                        emit_c2(a, b_, slot=slot)


            # --- tail: last dw chunk + last c2 chunk only ---
            for c in range(2):
                emit_dw(c, 5)
            for occ in range(2):
                emit_c2(occ, 5)

        if dbg:
            nc.gpsimd.dma_start(dbg["dq"][:], Q[:])
            nc.gpsimd.dma_start(dbg["dk"][:], K[:])
            nc.gpsimd.dma_start(dbg["dvt"][:], VTO[:])
            nc.sync.dma_start(dbg["dy0"][:], y_all[0][:])
            nc.sync.dma_start(dbg["dy1"][:], y_all[1][:])
            

def make_in_maps(x_full, consts):
    maps = []
    for b in range(8):
        m = dict(consts)
        m["x"] = np.ascontiguousarray(x_full[b].reshape(256, HW), dtype=np.float32)
        maps.append(m)
    return maps

_CACHED = {}


def _get_nc():
    if 'nc' not in _CACHED:
        _CACHED['nc'] = build_nc(debug=False)
    return _CACHED['nc']


def kernel(**inputs):
    """Full (unsharded) inputs -> full output (8, 256, 48, 48) float32."""
    from concourse.bass_utils import run_bass_kernel_spmd

    x = np.asarray(inputs['x'], dtype=np.float32)
    consts = build_consts(**{k: np.asarray(v) for k, v in inputs.items()
                             if k != 'x'})
    in_maps = make_in_maps(x, consts)
    nc = _get_nc()
    res = run_bass_kernel_spmd(nc, in_maps, list(range(8)))
    out = np.stack([res.results[b]['out'].reshape(256, 48, 48)
                    for b in range(8)])
    return out.astype(np.float32)

